# revision 20
# baseline (speedup 1.0000x reference)
"""Trainium2 Bass kernel for depthwise-multiplier conv + ReLU + per-out-channel
1x1 combine (nn_Comb_70016556859799).

Math (reference):
  out[b,o,p,q] = bc[o] + sum_i Wc[o,i] * relu( sum_{dy,dx} Wf[o,i,dy,dx]*x[b,i,p+dy,q+dx] + bf[o,i] )

Sharding: 8 cores = (batch b in 0..3) x (H half in 0..1). Each core computes
out[b, :, 63*h : 63*h+63, :] from x[b, :, 63*h : 63*h+65, :].

Per-core dataflow:
  - conv: z_i[o, pix] = Wf[:,i]^T(9x64) @ Xs_i(9 x pix) where Xs_i is a
    host-precomputed im2col layout (one clean strided DMA per channel). Four
    input channels run concurrently on the PE via 32-row/64-col tile packing.
  - relu+bias: ScalarE activation / VectorE tensor_scalar, PSUM->SBUF, bf16.
  - combine: out[o,pix] += Wc[o,i]*r_i[o,pix] as matmuls with stacked-diagonal
    lhsT (two channels per matmul, K=128) accumulated in PSUM; two chunks run
    concurrently via column-group packing.

Pixels are processed in 4-row x 128-col chunks (=512 f32 = one PSUM bank);
columns 126,127 are zero padding. Rows 0..62 are the real output rows; the
im2col buffer is padded to 64 rows so every chunk is uniform.
"""
import numpy as np
import ml_dtypes

import concourse.bass as bass
import concourse.mybir as mybir
from concourse import tile
from concourse.bass_utils import run_bass_kernel_spmd

BF16 = mybir.dt.bfloat16
F32 = mybir.dt.float32
npbf16 = ml_dtypes.bfloat16

B, FIN, FOUT, KK, H, W = 4, 64, 64, 3, 128, 128
HO, WO = H - KK + 1, W - KK + 1          # 126, 126
RPC = HO // 2                             # 63 output rows per core
HALO = RPC + KK - 1                       # 65 input rows per core
NQ = FIN // 4                             # 16 quads of input channels
XR, XC = 64, 128                          # padded im2col rows/cols per core
CGRPS = [0, 16, 32, 48]                   # 4 chunk groups x 16 rows


def _hoist_extra_waits(nc):
    """Walrus supports only one sync-wait command per instruction. Tile can
    emit several (multiple producer procs). Hoist all but the last wait onto
    fresh same-engine NoOp instructions placed immediately before -- the waits
    still execute on the same engine sequencer in the same order, so the
    synchronization semantics are unchanged."""
    import copy
    n_hoist = 0
    for blk in nc.m.functions[0].blocks:
        newinsts = []
        for inst in blk.instructions:
            si = getattr(inst, "sync_info", None)
            ow = list(si.on_wait) if si is not None and si.on_wait else []
            if len(ow) > 1:
                for wi, w in enumerate(ow[:-1]):
                    nop = mybir.InstNoOp(
                        name=f"{inst.name}_hw{wi}",
                        text_hint="hoisted_wait",
                        bass_nofuse=True,
                    )
                    nop.engine = inst.engine
                    nsi = copy.deepcopy(si)
                    nsi.on_wait = [w]
                    if getattr(nsi, "on_update", None):
                        nsi.on_update = []
                    nop.sync_info = nsi
                    newinsts.append(nop)
                    n_hoist += 1
                si.on_wait = [ow[-1]]
            newinsts.append(inst)
        blk.instructions = newinsts
    return n_hoist


def _build(hoist=True):
    nc = bass.Bass()
    xs_d = nc.declare_dram_parameter("xs", [FIN, 9, XR, XC], BF16, isOutput=False)
    wq_d = nc.declare_dram_parameter("wq", [NQ, 128, FOUT], BF16, isOutput=False)
    ds_d = nc.declare_dram_parameter("ds", [2 * NQ, 128, FOUT], BF16, isOutput=False)
    bfp_d = nc.declare_dram_parameter("bfp", [2 * NQ, 128, 1], F32, isOutput=False)
    bc2_d = nc.declare_dram_parameter("bc2", [128, 1], F32, isOutput=False)
    out_d = nc.declare_dram_parameter("out", [FOUT, RPC, XC], F32, isOutput=True)

    with tile.TileContext(nc) as tc:
        with (
            tc.tile_pool(name="wpool", bufs=1) as wpool,
            tc.tile_pool(name="xpool", bufs=3) as xpool,
            tc.tile_pool(name="rpool", bufs=6) as rpool,
            tc.tile_pool(name="opool", bufs=2) as opool,
            tc.tile_pool(name="psz", bufs=2, space=bass.MemorySpace.PSUM) as psz,
            tc.tile_pool(name="pso", bufs=4, space=bass.MemorySpace.PSUM) as pso,
        ):
            # resident weights
            wq_t = wpool.tile([128, NQ, FOUT], BF16, tag="wq")
            nc.sync.dma_start(wq_t[:], wq_d[:].transpose([1, 0, 2]))
            ds_t = wpool.tile([128, 2 * NQ, FOUT], BF16, tag="ds")
            nc.sync.dma_start(ds_t[:], ds_d[:].transpose([1, 0, 2]))
            bfp_t = wpool.tile([128, 2 * NQ], F32, tag="bfp")
            nc.sync.dma_start(bfp_t[:], bfp_d[:].transpose([1, 0, 2]).squeeze(2))
            bc2_t = wpool.tile([128, 1], F32, tag="bc2")
            nc.sync.dma_start(bc2_t[:], bc2_d[:])

            # align DMA queue round-robin phase to a multiple of 8 so the
            # recurring xs DMAs land on stable queues (no cross-queue WAW).
            pads_t = wpool.tile([1, 32], F32, tag="pads")
            for pi_ in range(4):
                nc.sync.dma_start(pads_t[0:1, pi_: pi_ + 1], bc2_d[0:1, 0:1])
            # warmups: each engine observes the weight-DMA semaphores once via
            # a tiny dummy op so real instructions never carry those waits.
            dummy = psz.tile([128, 2, 4, XC], F32, tag="z", name="zdummy")
            nc.tensor.matmul(
                dummy[0:64, 0, 0, 0:64], wq_t[0:9, 0, :], wq_t[0:9, 0, :],
                start=True, stop=True,
            )
            nc.tensor.matmul(
                dummy[0:64, 1, 0, 0:64], ds_t[:, 0, :], ds_t[:, 0, :],
                start=True, stop=True,
            )
            scr_a = wpool.tile([128, 1], F32, tag="scr_a")
            nc.scalar.activation(
                scr_a[:], bfp_t[:, 0:1],
                mybir.ActivationFunctionType.Relu, bias=bfp_t[:, 0:1],
            )
            scr_d1 = wpool.tile([128, 1], F32, tag="scr_d1")
            nc.vector.tensor_scalar(
                scr_d1[:], bc2_t[:], bc2_t[:, 0:1], None, mybir.AluOpType.add
            )
            scr_d2 = wpool.tile([128, 1], F32, tag="scr_d2")
            nc.vector.tensor_scalar(
                scr_d2[:], bfp_t[:, 0:1], bfp_t[:, 0:1], None, mybir.AluOpType.add
            )

            for gi, r0 in enumerate(CGRPS):
                # psum accumulators: po[0] holds chunks 0,1; po[1] chunks 2,3
                po = [pso.tile([128, 4, XC], F32, tag="po", name=f"po{gi}_{_pi}")
                      for _pi in range(2)]
                for q in range(NQ):
                    xs_t = xpool.tile([128, 16, XC], BF16, tag="xs")
                    for k in range(4):
                        # one contiguous DMA per channel (host im2col layout)
                        src = bass.AP(
                            xs_d,
                            (4 * q + k) * 9 * XR * XC + r0 * XC,
                            [[XR * XC, 9], [XC, 16], [1, XC]],
                        )
                        nc.sync.dma_start(xs_t[32 * k: 32 * k + 9, :, :], src)
                    for half in range(2):          # chunk pair (0,1) or (2,3)
                        for j in range(2):         # channel pair within quad
                            # conv: channel k=2j -> partitions 0:64 colgrp 0,
                            #       k=2j+1 -> partitions 64:128 colgrp 64
                            z_t = psz.tile([128, 2, 4, XC], F32, tag="z")
                            for ci in range(2):
                                c = 2 * half + ci
                                for kk_ in range(2):
                                    rg = 32 * (2 * j + kk_)
                                    nc.tensor.matmul(
                                        z_t[64 * kk_: 64 * kk_ + 64, ci, :, :],
                                        wq_t[rg: rg + 9, q, :],
                                        xs_t[rg: rg + 9, 4 * c: 4 * c + 4, :],
                                        start=True,
                                        stop=True,
                                        tile_position=(rg, 64 * kk_),
                                    )
                            # relu + per-partition bias -> bf16 SBUF
                            r_t = rpool.tile([128, 2, 4, XC], BF16, tag="r")
                            if j == 0:
                                nc.scalar.activation(
                                    r_t[:], z_t[:],
                                    mybir.ActivationFunctionType.Relu,
                                    bias=bfp_t[:, 2 * q + j: 2 * q + j + 1],
                                )
                            else:
                                nc.vector.tensor_scalar(
                                    r_t[:], z_t[:],
                                    bfp_t[:, 2 * q + j: 2 * q + j + 1], 0.0,
                                    mybir.AluOpType.add, mybir.AluOpType.max,
                                )
                            # combine: accumulate Wc-diag matmuls into po
                            for ci in range(2):
                                nc.tensor.matmul(
                                    po[half][64 * ci: 64 * ci + 64, :, :],
                                    ds_t[:, 2 * q + j, :],
                                    r_t[:, ci, :, :],
                                    start=(q == 0 and j == 0),
                                    stop=(q == NQ - 1 and j == 1),
                                    tile_position=(0, 64 * ci),
                                    skip_group_check=True,
                                )
                # evacuate: add bc, f32 out. Separate lo/hi tiles so each
                # absorber write soaks up exactly one out-DMA WAR semaphore;
                # the real evac ops then carry only the PE wait.
                obL = opool.tile([64, 2, 4, XC], F32, tag="obL")
                obH = opool.tile([128, 2, 4, XC], F32, tag="obH")
                nc.vector.tensor_scalar(
                    obL[0:1, 0, 0, 0:1], bc2_t[0:1, :], 0.0, None,
                    mybir.AluOpType.add,
                )
                nc.vector.tensor_scalar(
                    obH[64:65, 0, 0, 0:1], bc2_t[64:65, :], 0.0, None,
                    mybir.AluOpType.add,
                )
                for pi in range(2):
                    nc.vector.tensor_scalar(
                        obL[:, pi, :, :], po[pi][0:64, :, :], bc2_t[0:64, 0:1],
                        None, mybir.AluOpType.add,
                    )
                    nc.vector.tensor_scalar(
                        obH[64:128, pi, :, :], po[pi][64:128, :, :],
                        bc2_t[64:128, 0:1], None, mybir.AluOpType.add,
                    )
                # lo half: chunks 0,2 -> rows r0+{0..3, 8..11}
                dst_lo = bass.AP(
                    out_d, r0 * XC,
                    [[RPC * XC, FOUT], [8 * XC, 2], [XC, 4], [1, XC]],
                )
                nc.sync.dma_start(dst_lo, obL[:, :, :, :])
                if gi < 3:
                    dst_hi = bass.AP(
                        out_d, (r0 + 4) * XC,
                        [[RPC * XC, FOUT], [8 * XC, 2], [XC, 4], [1, XC]],
                    )
                    nc.sync.dma_start(dst_hi, obH[64:128, :, :, :])
                else:
                    dst_h1 = bass.AP(
                        out_d, (r0 + 4) * XC,
                        [[RPC * XC, FOUT], [XC, 4], [1, XC]],
                    )
                    nc.sync.dma_start(dst_h1, obH[64:128, 0, :, :])
                    dst_h3 = bass.AP(
                        out_d, (r0 + 12) * XC,
                        [[RPC * XC, FOUT], [XC, 3], [1, XC]],
                    )
                    nc.sync.dma_start(dst_h3, obH[64:128, 1, 0:3, :])
                n_out_dmas = 2 if gi < 3 else 3
                for pi_ in range(8 - n_out_dmas):
                    idx = 4 + gi * 6 + pi_
                    nc.sync.dma_start(pads_t[0:1, idx: idx + 1], bc2_d[0:1, 0:1])
    if hoist:
        _hoist_extra_waits(nc)
    return nc


_NC = None


def _get_nc():
    global _NC
    if _NC is None:
        _NC = _build()
    return _NC


def _pack_weights(Wf, bf, Wc, bc):
    Wf_t = Wf.transpose(1, 2, 3, 0).reshape(FIN, 9, FOUT)  # [i, t, o]
    wq = np.zeros((NQ, 128, FOUT), np.float32)
    for k in range(4):
        wq[:, 32 * k: 32 * k + 9, :] = Wf_t[np.arange(NQ) * 4 + k]
    ds = np.zeros((2 * NQ, 128, FOUT), np.float32)
    eye = np.eye(FOUT, dtype=np.float32)
    for p in range(2 * NQ):  # p = 2q+j ; channels (4q+2j, 4q+2j+1)
        q, j = p // 2, p % 2
        ds[p, 0:64, :] = eye * Wc[:, 4 * q + 2 * j][None, :]
        ds[p, 64:128, :] = eye * Wc[:, 4 * q + 2 * j + 1][None, :]
    bfp = np.zeros((2 * NQ, 128, 1), np.float32)
    for p in range(2 * NQ):
        q, j = p // 2, p % 2
        bfp[p, 0:64, 0] = bf[:, 4 * q + 2 * j]
        bfp[p, 64:128, 0] = bf[:, 4 * q + 2 * j + 1]
    bc2 = np.tile(bc.reshape(64, 1), (2, 1)).astype(np.float32)
    return {
        "wq": wq.astype(npbf16),
        "ds": ds.astype(npbf16),
        "bfp": bfp,
        "bc2": bc2,
    }


def _im2col(x, b, h):
    """[FIN, 9, XR, XC] bf16: xs[i, 3*dy+dx, r, c] = x[b, i, 63h+r+dy, c+dx]
    (zero-padded outside the valid range)."""
    xpad = np.zeros((FIN, XR + KK - 1, XC + KK - 1), np.float32)
    row_hi = min(H, RPC * h + XR + KK - 1)
    nrows = row_hi - RPC * h
    xpad[:, 0:nrows, 0:W] = x[b, :, RPC * h: row_hi, :]
    sw = np.lib.stride_tricks.sliding_window_view(xpad, (KK, KK), axis=(1, 2))
    return np.ascontiguousarray(
        sw.transpose(0, 3, 4, 1, 2).reshape(FIN, 9, XR, XC)
    ).astype(npbf16)


def _run(x, Wf, bf, Wc, bc, **spmd_kwargs):
    shared = _pack_weights(Wf, bf, Wc, bc)
    in_maps = []
    for core in range(8):
        b, h = core // 2, core % 2
        m = dict(shared)
        m["xs"] = _im2col(x, b, h)
        in_maps.append(m)
    res = run_bass_kernel_spmd(_get_nc(), in_maps, list(range(8)), **spmd_kwargs)
    out = np.empty((B, FOUT, HO, WO), np.float32)
    for core in range(8):
        b, h = core // 2, core % 2
        out[b, :, RPC * h: RPC * h + RPC, :] = np.asarray(
            res.results[core]["out"], np.float32
        )[:, :, 0:WO]
    return out, res


def kernel(x, Wf, bf, Wc, bc):
    x = np.asarray(x, np.float32)
    out, _ = _run(
        x,
        np.asarray(Wf, np.float32),
        np.asarray(bf, np.float32),
        np.asarray(Wc, np.float32),
        np.asarray(bc, np.float32),
    )
    return out
